# revision 16
# baseline (speedup 1.0000x reference)
"""Mamba block kernel for Trainium2, 8 NeuronCores.

Sharding: DP-2 over batch x TP-4 over d_inner (512 channels/core).
Core c = b*4 + g handles batch b, channels [g*512, (g+1)*512).

Per-core pipeline (everything in transposed [feature, time] layout):
  A) stats per L-half (mean/var over d_model via PE ones-reduce), in_proj
     (u and z halves) with LN folded in (rank-2 correction in PSUM),
     rstd scaling on DVE; silu(z) on ACT.
  B) depthwise causal conv on DVE (tensor_scalar taps + add tree), SiLU,
     x_proj partial + AllReduce(bf16, split by L-half), dt_proj +
     softplus, dt*u.
  C) selective scan per (dtile, n): dA = exp(A_n*dt) on ACT, dBu on DVE,
     h = tensor_tensor_scan chained over L-halves, hC on GPSIMD,
     n-accumulate + Dskip*u_c via PE diag/identity matmuls into PSUM.
     out_proj(h0) overlaps the final dtile's h1 scans.
  D) out_proj partial matmul -> bf16 partial output to HBM.

Host: preps transposed/bf16 weights, sums the 4 TP partials per batch
in f32, adds the residual.
"""

import numpy as np
import ml_dtypes

D_MODEL, D_STATE, D_CONV, EXPAND = 1024, 16, 4, 2
D_INNER = EXPAND * D_MODEL            # 2048
DT_RANK = 64
B, L = 2, 2048
EPS = 1e-5
N_CORES = 8
TP = 4                                # TP group size
DP = D_INNER // TP                    # 512 channels per core
NDT = DP // 128                       # 4 d-tiles per core
BF16 = ml_dtypes.bfloat16
LH = L // 2                           # half length

_CACHE = {}


def _build_program():
    import concourse.bass as bass
    import concourse.tile as tile
    from concourse import bacc, mybir

    F32, BF = mybir.dt.float32, mybir.dt.bfloat16
    ALU = mybir.AluOpType
    ACT = mybir.ActivationFunctionType

    nc = bacc.Bacc("TRN2", target_bir_lowering=False, debug=False,
                   num_devices=N_CORES)

    # ---- per-core external tensors ----
    xT = nc.dram_tensor("xT", [D_MODEL, L], BF, kind="ExternalInput")
    winT = nc.dram_tensor("winT", [D_MODEL, 2 * DP], BF, kind="ExternalInput")
    nsc = nc.dram_tensor("nsc", [2, 2 * DP], BF, kind="ExternalInput")
    convw = nc.dram_tensor("convw", [DP, D_CONV], F32, kind="ExternalInput")
    convb = nc.dram_tensor("convb", [DP, 1], F32, kind="ExternalInput")
    xpwT = nc.dram_tensor("xpwT", [DP, DT_RANK + 2 * D_STATE], BF, kind="ExternalInput")
    dtwT = nc.dram_tensor("dtwT", [DT_RANK, DP], BF, kind="ExternalInput")
    dtb = nc.dram_tensor("dtb", [DP, 1], F32, kind="ExternalInput")
    Aneg = nc.dram_tensor("Aneg", [DP, D_STATE], F32, kind="ExternalInput")
    dskd = nc.dram_tensor("dskd", [DP, 128], BF, kind="ExternalInput")
    ident_in = nc.dram_tensor("ident", [128, 128], BF, kind="ExternalInput")
    owT = nc.dram_tensor("owT", [DP, D_MODEL], BF, kind="ExternalInput")
    out = nc.dram_tensor("out", [D_MODEL, L], BF, kind="ExternalOutput")

    NK = D_MODEL // 128               # 8 k-chunks
    NXP = DT_RANK + 2 * D_STATE       # 96

    with tile.TileContext(nc) as tc:
        with tc.tile_pool(name="persist", bufs=1) as pp, \
             tc.tile_pool(name="dram", bufs=1, space="DRAM") as dram:

            # small persistent constants
            ident = pp.tile([128, 128], BF, tag="ident")
            nc.sync.dma_start(ident[:], ident_in.ap())
            dskd_sb, Aneg_sb, convw_sb, convb_sb, dtb_sb = [], [], [], [], []
            for i in range(NDT):
                rsl = slice(i * 128, (i + 1) * 128)
                t = pp.tile([128, 128], BF, name=f"dskd{i}")
                nc.sync.dma_start(t[:], dskd.ap()[rsl, :]); dskd_sb.append(t)
                t = pp.tile([128, D_STATE], F32, name=f"Aneg{i}")
                nc.sync.dma_start(t[:], Aneg.ap()[rsl, :]); Aneg_sb.append(t)
                t = pp.tile([128, D_CONV], F32, name=f"convw{i}")
                nc.sync.dma_start(t[:], convw.ap()[rsl, :]); convw_sb.append(t)
                t = pp.tile([128, 1], F32, name=f"convb{i}")
                nc.sync.dma_start(t[:], convb.ap()[rsl, :]); convb_sb.append(t)
                t = pp.tile([128, 1], F32, name=f"dtb{i}")
                nc.sync.dma_start(t[:], dtb.ap()[rsl, :]); dtb_sb.append(t)
            nsc_sb = pp.tile([2, 2 * DP], BF, tag="nsc")
            nc.sync.dma_start(nsc_sb[:], nsc.ap())
            ones = pp.tile([128, 1], BF, tag="ones")
            nc.vector.memset(ones[:], 1.0)
            s1sd = pp.tile([2, L], BF, tag="s1sd")
            rstd_bc = pp.tile([128, L], BF, tag="rstbc")

            # DRAM scratch
            xdbl_part = [dram.tile([NXP, LH], BF, tag=f"xdp{h}",
                                   name=f"xdp{h}") for h in range(2)]
            xdbl_red = [dram.tile([NXP, LH], BF, tag=f"xdr{h}",
                                  name=f"xdr{h}") for h in range(2)]
            rstd_dram = dram.tile([1, L], BF, tag="rstdd")

            # persistent activations that live into the scan phase
            ucT = [pp.tile([128, L], BF, tag=f"ucT{i}", name=f"ucT{i}")
                   for i in range(NDT)]
            szT = [pp.tile([128, L], BF, tag=f"szT{i}", name=f"szT{i}")
                   for i in range(NDT)]

            # ---------- Phase A (scoped pool: xk, uT, weights die after conv)
            with tc.tile_pool(name="uphase", bufs=1) as up:
                xk = [up.tile([128, L], BF, tag=f"xk{kc}", name=f"xk{kc}")
                      for kc in range(NK)]
                for kc in range(NK):
                    nc.sync.dma_start(xk[kc][:],
                                      xT.ap()[kc * 128:(kc + 1) * 128, :])
                uT = [up.tile([128, L + 4], BF, tag=f"uT{i}", name=f"uT{i}")
                      for i in range(NDT)]
                for i in range(NDT):
                    nc.vector.memset(uT[i][:, 0:4], 0.0)

                # stats per L-half
                with tc.tile_pool(name="stat", bufs=1) as statp:
                    epsb = statp.tile([1, 1], F32, tag="epsb")
                    nc.vector.memset(epsb[:], EPS)
                    with tc.tile_pool(name="stps", bufs=2, space="PSUM") as stps, \
                         tc.tile_pool(name="sq", bufs=2) as sqp:
                        for h2 in range(2):
                            hsl = slice(h2 * LH, (h2 + 1) * LH)
                            S1 = stps.tile([1, LH], F32, tag="S1")
                            S2 = stps.tile([1, LH], F32, tag="S2")
                            for kc in range(NK):
                                x2 = sqp.tile([128, LH], BF, tag="x2")
                                nc.scalar.activation(x2[:], xk[kc][:, hsl],
                                                     ACT.Square)
                                for t4 in range(2):
                                    sl = slice(t4 * 512, (t4 + 1) * 512)
                                    xsl = slice(h2 * LH + t4 * 512,
                                                h2 * LH + (t4 + 1) * 512)
                                    nc.tensor.matmul(S1[:, sl], ones[:],
                                                     xk[kc][:, xsl],
                                                     start=(kc == 0),
                                                     stop=(kc == NK - 1))
                                    nc.tensor.matmul(S2[:, sl], ones[:],
                                                     x2[:, sl],
                                                     start=(kc == 0),
                                                     stop=(kc == NK - 1))
                            s1f = statp.tile([1, LH], F32, tag="s1f", bufs=2)
                            s2f = statp.tile([1, LH], F32, tag="st", bufs=3)
                            nc.scalar.activation(s1f[:], S1[:], ACT.Copy)
                            nc.scalar.activation(s2f[:], S2[:], ACT.Copy)
                            mu2 = statp.tile([1, LH], F32, tag="st", bufs=3)
                            nc.scalar.activation(mu2[:], s1f[:], ACT.Square,
                                                 scale=1.0 / D_MODEL)
                            var = statp.tile([1, LH], F32, tag="st", bufs=3)
                            nc.vector.scalar_tensor_tensor(
                                var[:], s2f[:], 1.0 / D_MODEL, mu2[:],
                                ALU.mult, ALU.subtract)
                            lv = statp.tile([1, LH], F32, tag="st", bufs=3)
                            nc.scalar.activation(lv[:], var[:], ACT.Ln,
                                                 bias=epsb[:])
                            rstd_row = statp.tile([1, LH], BF, tag="st",
                                                  bufs=3)
                            nc.scalar.activation(rstd_row[:], lv[:], ACT.Exp,
                                                 scale=-0.5)
                            sd_row = statp.tile([1, LH], BF, tag="sdr", bufs=2)
                            nc.scalar.activation(sd_row[:], lv[:], ACT.Exp,
                                                 scale=0.5)
                            s1_row = statp.tile([1, LH], BF, tag="s1r", bufs=2)
                            nc.vector.tensor_copy(s1_row[:], s1f[:])
                            nc.sync.dma_start(s1sd[0:1, hsl], s1_row[:])
                            nc.sync.dma_start(s1sd[1:2, hsl], sd_row[:])
                            nc.sync.dma_start(rstd_dram[:, hsl], rstd_row[:])
                    nc.sync.dma_start(
                        rstd_bc[:], rstd_dram[0, :].partition_broadcast(128))

                # in_proj u-half then z-half (weights streamed per chunk)
                with tc.tile_pool(name="xzps", bufs=2, space="PSUM") as xzps, \
                     tc.tile_pool(name="wst", bufs=4) as wstp, \
                     tc.tile_pool(name="zev", bufs=1) as zevp:
                    for mcg in range(2 * NDT):
                        is_z = mcg >= NDT
                        mc = mcg - NDT if is_z else mcg
                        wcol = (DP if is_z else 0) + mc * 128
                        ps = xzps.tile([128, L], F32, tag="xz")
                        for kc in range(NK):
                            w = wstp.tile([128, 128], BF, tag="w")
                            nc.sync.dma_start(
                                w[:], winT.ap()[kc * 128:(kc + 1) * 128,
                                                wcol:wcol + 128])
                            for t4 in range(4):
                                sl = slice(t4 * 512, (t4 + 1) * 512)
                                nc.tensor.matmul(ps[:, sl], w[:],
                                                 xk[kc][:, sl],
                                                 start=(kc == 0), stop=False)
                        for t4 in range(4):
                            sl = slice(t4 * 512, (t4 + 1) * 512)
                            nc.tensor.matmul(
                                ps[:, sl], nsc_sb[:, wcol:wcol + 128],
                                s1sd[:, sl], start=False, stop=True)
                        if not is_z:
                            nc.vector.tensor_tensor(uT[mc][:, 4:4 + L], ps[:],
                                                    rstd_bc[:], ALU.mult)
                        else:
                            ztmp = zevp.tile([128, L], BF, tag="ztmp")
                            nc.vector.tensor_tensor(ztmp[:], ps[:],
                                                    rstd_bc[:], ALU.mult)
                            nc.scalar.activation(szT[mc][:], ztmp[:], ACT.Silu)

                # conv on DVE per half: 4 taps ts_mul + add tree; SiLU on ACT
                with tc.tile_pool(name="taps", bufs=1) as tapp:
                    for i in range(NDT):
                        for h in range(2):
                            o = h * LH
                            ca = tapp.tile([128, LH], BF, tag="ta", name="ca")
                            nc.vector.tensor_scalar_mul(
                                ca[:], uT[i][:, 1 + o:1 + o + LH],
                                convw_sb[i][:, 0:1])
                            cb = tapp.tile([128, LH], BF, tag="tb", name="cb")
                            nc.vector.tensor_scalar_mul(
                                cb[:], uT[i][:, 2 + o:2 + o + LH],
                                convw_sb[i][:, 1:2])
                            a01 = tapp.tile([128, LH], BF, tag="p0", name="a01")
                            nc.vector.tensor_tensor(a01[:], ca[:], cb[:],
                                                    ALU.add)
                            cc = tapp.tile([128, LH], BF, tag="ta", name="cc")
                            nc.vector.tensor_scalar_mul(
                                cc[:], uT[i][:, 3 + o:3 + o + LH],
                                convw_sb[i][:, 2:3])
                            cd = tapp.tile([128, LH], BF, tag="tb", name="cd")
                            nc.vector.tensor_scalar_mul(
                                cd[:], uT[i][:, 4 + o:4 + o + LH],
                                convw_sb[i][:, 3:4])
                            a23 = tapp.tile([128, LH], BF, tag="p1", name="a23")
                            nc.vector.tensor_tensor(a23[:], cc[:], cd[:],
                                                    ALU.add)
                            nc.vector.tensor_tensor(ucT[i][:, o:o + LH],
                                                    a01[:], a23[:], ALU.add)
                        nc.scalar.activation(ucT[i][:], ucT[i][:], ACT.Silu,
                                             bias=convb_sb[i][:])

            # ---------- Phase B: x_proj + AR + dt ----------
            xpw_t = []
            for i in range(NDT):
                t = pp.tile([128, NXP], BF, name=f"xpw{i}")
                nc.sync.dma_start(t[:], xpwT.ap()[i * 128:(i + 1) * 128, :])
                xpw_t.append(t)
            with tc.tile_pool(name="xpps", bufs=2, space="PSUM") as xpps, \
                 tc.tile_pool(name="xpe", bufs=2) as xpep:
                for h in range(2):
                    psx = xpps.tile([NXP, LH], F32, tag="xp")
                    for i in range(NDT):
                        for t4 in range(2):
                            sl = slice(h * LH + t4 * 512,
                                       h * LH + (t4 + 1) * 512)
                            osl = slice(t4 * 512, (t4 + 1) * 512)
                            nc.tensor.matmul(psx[:, osl], xpw_t[i][:],
                                             ucT[i][:, sl],
                                             start=(i == 0),
                                             stop=(i == NDT - 1))
                    xde = xpep.tile([NXP, LH], BF, tag="xde")
                    nc.scalar.activation(xde[:], psx[:], ACT.Copy)
                    nc.sync.dma_start(xdbl_part[h][:], xde[:])
                    nc.gpsimd.collective_compute(
                        "AllReduce", ALU.add,
                        replica_groups=[[0, 1, 2, 3], [4, 5, 6, 7]],
                        ins=[xdbl_part[h][:].opt()],
                        outs=[xdbl_red[h][:].opt()],
                    )

            dtT = [pp.tile([128, L], BF, tag=f"dtT{i}", name=f"dtT{i}")
                   for i in range(NDT)]
            dtuT = [pp.tile([128, L], BF, tag=f"dtuT{i}", name=f"dtuT{i}")
                    for i in range(NDT)]
            dtw_sb = pp.tile([DT_RANK, DP], BF, tag="dtw")
            nc.sync.dma_start(dtw_sb[:], dtwT.ap())
            dtr16 = pp.tile([DT_RANK, L], BF, tag="dtr16")
            for h in range(2):
                hsl = slice(h * LH, (h + 1) * LH)
                nc.sync.dma_start(dtr16[:, hsl], xdbl_red[h][0:DT_RANK, :])
            with tc.tile_pool(name="dtps", bufs=2, space="PSUM") as dtps, \
                 tc.tile_pool(name="dte", bufs=2) as dtep:
                for i in range(NDT):
                    psd = dtps.tile([128, L], F32, tag="dt")
                    for t4 in range(4):
                        sl = slice(t4 * 512, (t4 + 1) * 512)
                        nc.tensor.matmul(psd[:, sl],
                                         dtw_sb[:, i * 128:(i + 1) * 128],
                                         dtr16[:, sl], start=True, stop=True)
                    etile = dtep.tile([128, L], F32, tag="et")
                    nc.scalar.activation(etile[:], psd[:], ACT.Exp,
                                         bias=dtb_sb[i][:])
                    nc.scalar.activation(dtT[i][:], etile[:], ACT.Ln, bias=1.0)
                    nc.vector.tensor_tensor(dtuT[i][:], dtT[i][:],
                                            ucT[i][:], ALU.mult)

            # ---------- Phase C: selective scan ----------
            ysg = [pp.tile([128, L], BF, tag=f"ysg{i}", name=f"ysg{i}")
                   for i in range(NDT)]
            hstate = pp.tile([128, D_STATE], BF, tag="hstate")

            with tc.tile_pool(name="ysps", bufs=1, space="PSUM") as ysps, \
                 tc.tile_pool(name="scw", bufs=2) as scw, \
                 tc.tile_pool(name="bcw", bufs=2) as bcw, \
                 tc.tile_pool(name="ops", bufs=2, space="PSUM") as ops, \
                 tc.tile_pool(name="owp", bufs=2) as owp, \
                 tc.tile_pool(name="oev", bufs=2) as oevp:

                _LASTCOL = [None]

                def scan_dtile(i, ys, halves):
                    for n in range(D_STATE):
                        dA = scw.tile([128, L], BF, tag="dA")
                        nc.scalar.activation(dA[:], dtT[i][:], ACT.Exp,
                                             scale=Aneg_sb[i][:, n:n + 1])
                        for h in halves:
                            hsl = slice(h * LH, (h + 1) * LH)
                            bbc = bcw.tile([128, LH], BF, tag="bbc")
                            nc.sync.dma_start(
                                bbc[:], xdbl_red[h][DT_RANK + n, :]
                                .partition_broadcast(128))
                            cbc = bcw.tile([128, LH], BF, tag="cbc")
                            nc.sync.dma_start(
                                cbc[:], xdbl_red[h][DT_RANK + D_STATE + n, :]
                                .partition_broadcast(128))
                            dBu = scw.tile([128, LH], BF, tag="dBu")
                            nc.vector.tensor_tensor(dBu[:], dtuT[i][:, hsl],
                                                    bbc[:], ALU.mult)
                            hh = scw.tile([128, LH], BF, tag=f"hh{h}",
                                          name=f"hh{h}")
                            if h == 0:
                                init = 0.0
                            elif halves == (0, 1):
                                init = _LASTCOL[0][:, LH - 1:LH]
                            else:
                                init = hstate[:, n:n + 1]
                            nc.vector.tensor_tensor_scan(hh[:], dA[:, hsl],
                                                         dBu[:], init,
                                                         ALU.mult, ALU.add)
                            if halves == (0, 1) and h == 0:
                                _LASTCOL[0] = hh
                            if halves == (0,):
                                nc.scalar.activation(hstate[:, n:n + 1],
                                                     hh[:, LH - 1:LH],
                                                     ACT.Copy)
                            hC = scw.tile([128, LH], BF, tag="hC")
                            nc.gpsimd.tensor_tensor(hC[:], hh[:], cbc[:],
                                                    ALU.mult)
                            last = (n == D_STATE - 1)
                            for t4 in range(2):
                                sl = slice(t4 * 512, (t4 + 1) * 512)
                                nc.tensor.matmul(ys[h][:, sl], ident[:],
                                                 hC[:, sl], start=False,
                                                 stop=last)

                def gate(i, ys, halves):
                    for h in halves:
                        hsl = slice(h * LH, (h + 1) * LH)
                        nc.vector.tensor_tensor(ysg[i][:, hsl], ys[h][:],
                                                szT[i][:, hsl], ALU.mult)

                def out_proj(h):
                    hsl0 = h * LH
                    for mc in range(D_MODEL // 128):
                        ow_mc = []
                        for i in range(NDT):
                            w = owp.tile([128, 128], BF, tag=f"oww{i}",
                                         name=f"oww{i}")
                            nc.sync.dma_start(
                                w[:], owT.ap()[i * 128:(i + 1) * 128,
                                               mc * 128:(mc + 1) * 128])
                            ow_mc.append(w)
                        for t4 in range(2):
                            sl = slice(hsl0 + t4 * 512, hsl0 + (t4 + 1) * 512)
                            po = ops.tile([128, 512], F32, tag="po")
                            for i in range(NDT):
                                nc.tensor.matmul(
                                    po[:], ow_mc[i][:], ysg[i][:, sl],
                                    start=(i == 0), stop=(i == NDT - 1))
                            oe = oevp.tile([128, 512], BF, tag="oe")
                            nc.scalar.activation(oe[:], po[:], ACT.Copy)
                            nc.sync.dma_start(
                                out.ap()[mc * 128:(mc + 1) * 128, sl], oe[:])

                for i in range(NDT):
                    ys = {}
                    for h in range(2):
                        ys[h] = ysps.tile([128, LH], F32, tag=f"ys{h}",
                                          name=f"ys{h}")
                        for t4 in range(2):
                            sl = slice(t4 * 512, (t4 + 1) * 512)
                            hsl = slice(h * LH + t4 * 512,
                                        h * LH + (t4 + 1) * 512)
                            nc.tensor.matmul(ys[h][:, sl], dskd_sb[i][:],
                                             ucT[i][:, hsl], start=True,
                                             stop=False)
                    if i < NDT - 1:
                        scan_dtile(i, ys, (0, 1))
                        gate(i, ys, (0, 1))
                    else:
                        scan_dtile(i, ys, (0,))
                        gate(i, ys, (0,))
                        out_proj(0)
                        scan_dtile(i, ys, (1,))
                        gate(i, ys, (1,))
                        out_proj(1)

    nc.compile()
    return nc


def _prep_inputs(x, ln_w, ln_b, in_proj_w, conv_w, conv_b, x_proj_w,
                 dt_proj_w, dt_proj_b, A_log, Dskip, out_proj_w):
    """Host-side shard + transpose + dtype prep. Returns list of 8 in_maps."""
    f32 = np.float32
    x = np.asarray(x, f32)
    ln_w = np.asarray(ln_w, f32); ln_b = np.asarray(ln_b, f32)
    W = np.asarray(in_proj_w, f32)
    W_eff = W * ln_w[None, :]
    c0 = W @ ln_b                                  # [2*D_INNER]
    rs = W_eff.sum(axis=1)                         # [2*D_INNER]
    A = -np.exp(np.asarray(A_log, f32))            # [D_INNER, 16]
    conv_w = np.asarray(conv_w, f32).reshape(D_INNER, D_CONV)
    conv_b = np.asarray(conv_b, f32)
    xpw = np.asarray(x_proj_w, f32)                # [96, D_INNER]
    dtw = np.asarray(dt_proj_w, f32)               # [D_INNER, 64]
    dtb = np.asarray(dt_proj_b, f32)
    Dsk = np.asarray(Dskip, f32)
    Ow = np.asarray(out_proj_w, f32)               # [D_MODEL, D_INNER]
    ident = np.eye(128, dtype=BF16)

    in_maps = []
    for c in range(N_CORES):
        b, g = divmod(c, TP)
        dsl = slice(g * DP, (g + 1) * DP)
        u_rows = slice(g * DP, (g + 1) * DP)
        z_rows = slice(D_INNER + g * DP, D_INNER + (g + 1) * DP)
        winT = np.concatenate([W_eff[u_rows].T, W_eff[z_rows].T], axis=1)
        negrs_c = -np.concatenate([rs[u_rows], rs[z_rows]]) / D_MODEL
        c0_c = np.concatenate([c0[u_rows], c0[z_rows]])
        nsc_c = np.stack([negrs_c, c0_c], axis=0)  # [2, 2*DP]
        dskd = np.zeros((DP, 128), BF16)
        for i in range(NDT):
            blk = np.diag(Dsk[g * DP + i * 128: g * DP + (i + 1) * 128])
            dskd[i * 128:(i + 1) * 128, :] = blk.astype(BF16)
        in_maps.append({
            "xT": np.ascontiguousarray(x[b].T).astype(BF16),
            "winT": winT.astype(BF16),
            "nsc": nsc_c.astype(BF16),
            "convw": np.ascontiguousarray(conv_w[dsl]),
            "convb": conv_b[dsl][:, None].copy(),
            "xpwT": np.ascontiguousarray(xpw[:, dsl].T).astype(BF16),
            "dtwT": np.ascontiguousarray(dtw[dsl].T).astype(BF16),
            "dtb": dtb[dsl][:, None].copy(),
            "Aneg": np.ascontiguousarray(A[dsl]),
            "dskd": dskd,
            "ident": ident,
            "owT": np.ascontiguousarray(Ow[:, dsl].T).astype(BF16),
        })
    return in_maps


def kernel(**inputs):
    from concourse.bass_utils import run_bass_kernel_spmd

    if "nc" not in _CACHE:
        _CACHE["nc"] = _build_program()
    nc = _CACHE["nc"]

    in_maps = _prep_inputs(**inputs)
    res = run_bass_kernel_spmd(nc, in_maps, core_ids=list(range(N_CORES)))

    x = np.asarray(inputs["x"], np.float32)
    out = np.empty((B, L, D_MODEL), np.float32)
    for b in range(B):
        acc = res.results[4 * b]["out"].astype(np.float32)
        for g in range(1, TP):
            acc += res.results[4 * b + g]["out"].astype(np.float32)
        out[b] = acc.T + x[b]
    return out
